# revision 12
# baseline (speedup 1.0000x reference)
"""BatchHardTripletLoss on 8 Trainium2 NeuronCores — v2.

Changes vs v1 baseline:
  - colterm is folded INTO the gram matmul: embeddings are rotated by the
    PCA basis of E^T E and the two least-variance dimensions are replaced
    by two "colterm channels" (16*q1 + q2 = -colterm), so PSUM holds
      v = 2 e_m.e_n - colterm_n  (+small noise)
    and mining needs no second full-size operand stream.
  - hardest-negative mining via the TENSOR_MASK_REDUCE custom-DVE op with
    per-row [lo,hi) wrap-inverted band masks:  acc = max(v outside band),
    hn_raw = -acc.  Full-range variant for the band-free column group.
  - hardest-positive via a small SUB_MAX custom op over the 256-wide
    band window:  acc = max(bump - v) = max over band of w'.
  - input DMA drops the 2.5MB ctmin/ctg1 tiles (now ~0.26MB of window
    bump tiles + lo/hi scalars).
"""

import dataclasses

import numpy as np
import ml_dtypes

import concourse.bacc as bacc
import concourse.mybir as mybir
from concourse.bass_utils import run_bass_kernel_spmd
from concourse.tile import TileContext
from concourse import dve_ops as _dve_ops
from concourse.dve_spec import (
    AluOp, C0, C1, C2, C3, Idx, Spec, Src0, Src1, Zero,
    _spill_c3_to_src1, lower, select,
)
from concourse.dve_uop import DveOpSpec


def _register_op(name, spec):
    for op in _dve_ops.OPS:
        if op.name == name:
            return op
    op = _dve_ops.DveOp(name, spec, subdim=False, uops_sha={})
    _dve_ops.OPS.append(op)
    opcode = _dve_ops._CUSTOM_DVE_ROW_BASE + len(_dve_ops.OPS) - 1
    assert opcode < 0x20
    _dve_ops._SUB_OPCODE_FOR_NAME[name] = opcode
    _dve_ops.CUSTOM_DVE_SPECS[name] = spec
    shas = {}
    for ver in ("v3", "v4"):
        s = DveOpSpec(name=name, opcode=opcode, uops=lower(spec, ver=ver),
                      rd1_en=True)
        shas[ver] = s.sha(ver)
    op = dataclasses.replace(op, uops_sha=shas)
    _dve_ops.OPS[-1] = op
    return op


ADD_MAX_OP = _register_op(
    "ANT_ADD_MAX",
    Spec(
        body=Src0 + Src1,
        accum=AluOp.MAX,
        accum_init=C2,
        reference=lambda in0, in1, s0, s1, imm2: (in0 + in1).astype(np.float32),
    ),
)


def _band_min_ref(in0, in1, s0, s1, imm2):
    p, n = in0.shape[0], in0.reshape(in0.shape[0], -1).shape[1]
    idx = np.arange(n, dtype=np.float32)[None, :]
    hi = np.asarray(in1, np.float32).reshape(p, 1)
    lo = np.asarray(s0, np.float32).reshape(-1, 1) * np.ones((p, 1), np.float32)
    pen = np.where((idx >= lo) & (idx < hi), np.float32(imm2), np.float32(0.0))
    return (in0.reshape(p, -1).astype(np.float32) + pen)


# acc = min over j of (in0[p,j] + imm2*[lo[p] <= j < hi[p]]); init s1
BAND_MIN_OP = _register_op(
    "ANT_BAND_MIN",
    Spec(
        body=_spill_c3_to_src1(
            Src0 + select((Idx >= C0) & (Idx < C3), C2, Zero)),
        accum=AluOp.MIN,
        accum_init=C1,
        reference=_band_min_ref,
    ),
)

# acc = min over j of (in0[p,j] + in1[p,0]); init imm2 (pure min w/ in1=0)
ADD_MIN_OP = _register_op(
    "ANT_ADD_MIN",
    Spec(
        body=Src0 + Src1,
        accum=AluOp.MIN,
        accum_init=C2,
        reference=lambda in0, in1, s0, s1, imm2: (in0 + in1).astype(np.float32),
    ),
)


def _band_min2_ref(in0, in1, s0, s1, imm2):
    p = in0.shape[0]
    x = in0.reshape(p, -1).astype(np.float32)
    n = x.shape[1]
    idx = np.arange(n, dtype=np.float32)[None, :]
    lo = np.asarray(s0, np.float32).reshape(-1, 1) * np.ones((p, 1), np.float32)
    hi = np.asarray(s1, np.float32).reshape(-1, 1) * np.ones((p, 1), np.float32)
    pen = np.where((idx >= lo) & (idx < hi), np.float32(imm2), np.float32(0.0))
    return x + pen


# acc = min over j of (in0[p,j] + imm2*[s0[p] <= j < s1[p]]); init imm2.
# No Src1 stream -> no latch-init uop (saves ~58ns/op vs ANT_BAND_MIN).
BAND_MIN2_OP = _register_op(
    "ANT_BAND_MIN2",
    Spec(
        body=Src0 + select((Idx >= C0) & (Idx < C1), C2, Zero),
        accum=AluOp.MIN,
        accum_init=C2,
        reference=_band_min2_ref,
    ),
)

B = 4096          # batch (anchors)
D = 512           # embedding dim
N_CORES = 8
ROWS = B // N_CORES      # 512 anchor rows per core
P = 128                  # partitions
MT = ROWS // P           # 4 m-tiles per core
NW = 512                 # psum bank width (fp32)
GW = 2048                # column group width (4 banks)
NG = B // GW             # 2 column groups
KT = D // P              # 4 contraction k-tiles (paired 2x for DoubleRow)

MARGIN = 0.5
EPS = 1e-6
BIG = 65536.0
AW = 256                 # band-max window width per m-tile
WLO = [0, 64, 192, 320]  # band window start per m-tile
WARM_MM = 72             # PE warmup matmuls (HAM ramp + input DMA cover)

_nc_cache = {}


def _build():
    nc = bacc.Bacc("TRN2", target_bir_lowering=False)
    fp16 = mybir.dt.float16
    fp8 = mybir.dt.float8e4
    f32 = mybir.dt.float32
    DR = mybir.MatmulPerfMode.DoubleRow

    et = nc.dram_tensor("et", [D, B], fp8, kind="ExternalInput")
    eblk = nc.dram_tensor("eblk", [D, ROWS], fp8, kind="ExternalInput")
    lohi = nc.dram_tensor("lohi", [P, 2 * MT], f32, kind="ExternalInput")
    outa = nc.dram_tensor("outa", [P, 2 * MT], f32, kind="ExternalOutput")
    outb = nc.dram_tensor("outb", [P, MT], f32, kind="ExternalOutput")

    with TileContext(nc) as tc:
        with (
            tc.tile_pool(name="etp", bufs=1) as etp,
            tc.tile_pool(name="wp", bufs=2) as wp,
            tc.tile_pool(name="accp", bufs=2) as accp,
            tc.tile_pool(name="psp", bufs=2, space="PSUM") as psp,
        ):
            # PE warmup: keeps the HAM clock ramping while input DMAs land
            warm = etp.tile([P, 96], fp16, tag="warm")
            nc.vector.memset(warm, 0.0)
            wps = psp.tile([P, GW], f32, tag="ps", name="wps")
            for _ in range(WARM_MM):
                nc.tensor.matmul(wps[:64, 0:96], warm[:, 0:64], warm[:, 0:96],
                                 start=True, stop=True)

            eb_all = etp.tile([P, KT * ROWS], fp8, tag="eb", name="eb_all")
            et_all = etp.tile([P, KT * B], fp8, tag="et", name="et_all")
            eb4 = eb_all.rearrange("p (k n) -> p k n", k=KT)
            et4 = et_all.rearrange("p (k n) -> p k n", k=KT)
            ebd4 = eblk.rearrange("(k p) n -> p k n", p=P)
            etd4 = et.rearrange("(k p) n -> p k n", p=P)
            lohi_sb = etp.tile([P, 2 * MT], f32, tag="lohi")

            # head DMAs: descriptor issue is ~0.8-0.9us each when serialized
            # on one engine, and the first block is DMA-paced. Fan the four
            # group-0 et pieces out across idle engine queues (vector/
            # scalar/sync) so their transfers start concurrently, ordered by
            # matmul consumption (u0 eats all of k01 first).
            H = GW // 2
            nc.gpsimd.dma_start(out=eb4, in_=ebd4)
            nc.sync.dma_start(out=et4[:, 0:2, 0:H], in_=etd4[:, 0:2, 0:H])
            nc.scalar.dma_start(out=et4[:, 0:2, H:GW], in_=etd4[:, 0:2, H:GW])
            nc.sync.dma_start(out=et4[:, 2:4, 0:H], in_=etd4[:, 2:4, 0:H])
            nc.gpsimd.dma_start(out=et4[:, 2:4, H:GW], in_=etd4[:, 2:4, H:GW])
            nc.gpsimd.dma_start(out=lohi_sb, in_=lohi[:, :])
            nc.gpsimd.dma_start(out=et4[:, 0:2, GW:B], in_=etd4[:, 0:2, GW:B])
            nc.gpsimd.dma_start(out=et4[:, 2:4, GW:B], in_=etd4[:, 2:4, GW:B])

            osb = accp.tile([P, 3 * MT], f32, tag="osb", name="osb")
            for g in range(NG):
                for t in range(MT):
                    ms = slice(t * P, (t + 1) * P)
                    ps = psp.tile([P, GW], f32, tag="ps", name="ps")
                    # gram: w' = -2 e_m.e_n + colterm_n (k-major, fp8 DR)
                    for u in range(2):
                        for j in range(GW // NW):
                            cs = slice(g * GW + j * NW, g * GW + (j + 1) * NW)
                            js = slice(j * NW, (j + 1) * NW)
                            nc.tensor.matmul(
                                ps[:, js],
                                eb4[:, 2 * u:2 * u + 2, ms],
                                et4[:, 2 * u:2 * u + 2, cs],
                                start=(u == 0), stop=(u == 1),
                                perf_mode=DR,
                            )
                    scr = wp.tile([P, GW], f32, tag="scr", name="scr")
                    if g == 0:
                        # hardest negative: min of w' + BIG*[lo<=j<hi);
                        # the out stream (w' + BIG*band) doubles as the
                        # hardest-positive input below
                        nc.vector._custom_dve(
                            BAND_MIN2_OP,
                            out=scr,
                            in0=ps,
                            s0=lohi_sb[:, t:t + 1],             # lo
                            s1=lohi_sb[:, MT + t:MT + t + 1],   # hi
                            imm2=BIG,
                            accum_out=osb[:, MT + t:MT + t + 1],
                        )
                        # hardest positive: max over band of w' =
                        # max(scr window) - BIG (band entries dominate)
                        nc.vector.tensor_reduce(
                            osb[:, t:t + 1],
                            scr[:, WLO[t]:WLO[t] + AW],
                            axis=mybir.AxisListType.X,
                            op=mybir.AluOpType.max,
                        )
                    else:
                        # hardest negative, group 1: plain min (empty band)
                        nc.vector._custom_dve(
                            BAND_MIN2_OP,
                            out=scr,
                            in0=ps,
                            s0=0.0,
                            s1=0.0,               # empty band -> plain min
                            imm2=BIG,
                            accum_out=osb[:, 2 * MT + t:2 * MT + t + 1],
                        )
                if g == 0:
                    nc.gpsimd.dma_start(out=outa[:, :], in_=osb[:, 0:2 * MT])
            nc.gpsimd.dma_start(out=outb[:, :], in_=osb[:, 2 * MT:3 * MT])
    nc.compile()
    return nc


def _get_nc():
    if "nc" not in _nc_cache:
        _nc_cache["nc"] = _build()
    return _nc_cache["nc"]


def _prepare_inputs(embeddings, labels):
    f8 = ml_dtypes.float8_e4m3
    Ef = np.ascontiguousarray(np.asarray(embeddings, dtype=np.float32))
    lab = np.asarray(labels).astype(np.int64)
    perm = np.argsort(lab, kind="stable")
    Ef = Ef[perm]
    labp = lab[perm]

    sq = np.sum(Ef * Ef, axis=1, dtype=np.float32)          # [B]
    s = np.sum(Ef, axis=1, dtype=np.float32)                # [B]
    rowterm = (sq + 2.0 * EPS * s + D * EPS * EPS).astype(np.float32)
    colterm = (sq - 2.0 * EPS * s).astype(np.float32)

    # rotate embeddings; drop the 2 least-variance dims for colterm channels
    G = (Ef.T @ Ef).astype(np.float64)
    _, V = np.linalg.eigh(G)                                # ascending
    F = (Ef @ V[:, 2:].astype(np.float32))                  # [B, 510]

    # colterm channels: 16*q1 + q2 == +colterm (+|err| <= ~0.5)
    q1 = (colterm / 16.0).astype(f8)
    q2 = (colterm - 16.0 * q1.astype(np.float32)).astype(f8)

    rt2 = np.float32(np.sqrt(2.0))
    X = np.empty((D, B), dtype=f8)                          # moving
    X[0:D - 2] = (F.T * rt2).astype(f8)
    X[D - 2] = q1
    X[D - 1] = q2
    Y = np.empty((D, B), dtype=f8)                          # stationary
    Y[0:D - 2] = (F.T * -rt2).astype(f8)
    Y[D - 2] = np.float32(16.0)
    Y[D - 1] = np.float32(1.0)

    seg_start = np.searchsorted(labp, labp, side="left")
    seg_end = np.searchsorted(labp, labp, side="right")

    jj = np.arange(AW)
    in_maps = []
    for c in range(N_CORES):
        r0, r1 = c * ROWS, (c + 1) * ROWS
        w0 = int(seg_start[r0])
        lo_b = (seg_start[r0:r1] - w0).astype(np.float32)
        hi_b = (seg_end[r0:r1] - w0).astype(np.float32)
        colperm = (np.arange(B) + w0) % B

        lohi_a = np.empty((P, 2 * MT), dtype=np.float32)
        for t in range(MT):
            tl = lo_b[t * P:(t + 1) * P][:, None]
            th = hi_b[t * P:(t + 1) * P][:, None]
            assert tl.min() >= WLO[t] and th.max() <= WLO[t] + AW, (
                c, t, tl.min(), th.max())
            lohi_a[:, t] = lo_b[t * P:(t + 1) * P]
            lohi_a[:, MT + t] = hi_b[t * P:(t + 1) * P]

        in_maps.append({
            "et": np.ascontiguousarray(X[:, colperm]),
            "eblk": np.ascontiguousarray(Y[:, r0:r1]),
            "lohi": lohi_a,
        })
    return in_maps, labp, rowterm


def _extract_hphn(results):
    """Return (hp_raw, hn_raw) full-length arrays from device results."""
    hp_l, hn_l = [], []
    for r in results:
        a = r["outa"]                                     # [P, 2*MT]
        b = r["outb"]                                     # [P, MT]
        hp_l.append((a[:, 0:MT] - np.float32(65536.0)).T.reshape(-1))
        hn_l.append(np.minimum(a[:, MT:2 * MT], b).T.reshape(-1))
    return np.concatenate(hp_l), np.concatenate(hn_l)


def _postprocess(results, labp, rowterm):
    hp_raw, hn_raw = _extract_hphn(results)
    hp2 = hp_raw + rowterm
    hn2 = hn_raw + rowterm
    hp = np.sqrt(np.maximum(hp2, 0.0, dtype=np.float32))
    hn = np.sqrt(np.maximum(hn2, 0.0, dtype=np.float32))

    cnt_lab = np.bincount(labp, minlength=1)
    n_same = cnt_lab[labp]
    valid = (n_same > 1) & (n_same < B)
    per = np.where(valid, np.maximum(hp - hn + np.float32(MARGIN), 0.0), 0.0)
    cnt = np.float32(valid.sum())
    if cnt > 0:
        loss = np.float32(per.sum(dtype=np.float32) / max(cnt, np.float32(1.0)))
    else:
        loss = np.float32(0.0)
    return np.asarray(loss, dtype=np.float32)


def _run(in_maps, **kw):
    nc = _get_nc()
    return run_bass_kernel_spmd(nc, in_maps, core_ids=list(range(N_CORES)), **kw)


def kernel(embeddings, labels):
    in_maps, labp, rowterm = _prepare_inputs(embeddings, labels)
    res = _run(in_maps)
    return _postprocess(res.results, labp, rowterm)


# revision 13
# speedup vs baseline: 1.0118x; 1.0118x over previous
"""BatchHardTripletLoss on 8 Trainium2 NeuronCores — v2.

Changes vs v1 baseline:
  - colterm is folded INTO the gram matmul: embeddings are rotated by the
    PCA basis of E^T E and the two least-variance dimensions are replaced
    by two "colterm channels" (16*q1 + q2 = -colterm), so PSUM holds
      v = 2 e_m.e_n - colterm_n  (+small noise)
    and mining needs no second full-size operand stream.
  - hardest-negative mining via the TENSOR_MASK_REDUCE custom-DVE op with
    per-row [lo,hi) wrap-inverted band masks:  acc = max(v outside band),
    hn_raw = -acc.  Full-range variant for the band-free column group.
  - hardest-positive via a small SUB_MAX custom op over the 256-wide
    band window:  acc = max(bump - v) = max over band of w'.
  - input DMA drops the 2.5MB ctmin/ctg1 tiles (now ~0.26MB of window
    bump tiles + lo/hi scalars).
"""

import dataclasses

import numpy as np
import ml_dtypes

import concourse.bacc as bacc
import concourse.mybir as mybir
from concourse.bass_utils import run_bass_kernel_spmd
from concourse.tile import TileContext
from concourse import dve_ops as _dve_ops
from concourse.dve_spec import (
    AluOp, C0, C1, C2, C3, Idx, Spec, Src0, Src1, Zero,
    _spill_c3_to_src1, lower, select,
)
from concourse.dve_uop import DveOpSpec


def _register_op(name, spec):
    for op in _dve_ops.OPS:
        if op.name == name:
            return op
    op = _dve_ops.DveOp(name, spec, subdim=False, uops_sha={})
    _dve_ops.OPS.append(op)
    opcode = _dve_ops._CUSTOM_DVE_ROW_BASE + len(_dve_ops.OPS) - 1
    assert opcode < 0x20
    _dve_ops._SUB_OPCODE_FOR_NAME[name] = opcode
    _dve_ops.CUSTOM_DVE_SPECS[name] = spec
    shas = {}
    for ver in ("v3", "v4"):
        s = DveOpSpec(name=name, opcode=opcode, uops=lower(spec, ver=ver),
                      rd1_en=True)
        shas[ver] = s.sha(ver)
    op = dataclasses.replace(op, uops_sha=shas)
    _dve_ops.OPS[-1] = op
    return op


ADD_MAX_OP = _register_op(
    "ANT_ADD_MAX",
    Spec(
        body=Src0 + Src1,
        accum=AluOp.MAX,
        accum_init=C2,
        reference=lambda in0, in1, s0, s1, imm2: (in0 + in1).astype(np.float32),
    ),
)


def _band_min_ref(in0, in1, s0, s1, imm2):
    p, n = in0.shape[0], in0.reshape(in0.shape[0], -1).shape[1]
    idx = np.arange(n, dtype=np.float32)[None, :]
    hi = np.asarray(in1, np.float32).reshape(p, 1)
    lo = np.asarray(s0, np.float32).reshape(-1, 1) * np.ones((p, 1), np.float32)
    pen = np.where((idx >= lo) & (idx < hi), np.float32(imm2), np.float32(0.0))
    return (in0.reshape(p, -1).astype(np.float32) + pen)


# acc = min over j of (in0[p,j] + imm2*[lo[p] <= j < hi[p]]); init s1
BAND_MIN_OP = _register_op(
    "ANT_BAND_MIN",
    Spec(
        body=_spill_c3_to_src1(
            Src0 + select((Idx >= C0) & (Idx < C3), C2, Zero)),
        accum=AluOp.MIN,
        accum_init=C1,
        reference=_band_min_ref,
    ),
)

# acc = min over j of (in0[p,j] + in1[p,0]); init imm2 (pure min w/ in1=0)
ADD_MIN_OP = _register_op(
    "ANT_ADD_MIN",
    Spec(
        body=Src0 + Src1,
        accum=AluOp.MIN,
        accum_init=C2,
        reference=lambda in0, in1, s0, s1, imm2: (in0 + in1).astype(np.float32),
    ),
)

B = 4096          # batch (anchors)
D = 512           # embedding dim
N_CORES = 8
ROWS = B // N_CORES      # 512 anchor rows per core
P = 128                  # partitions
MT = ROWS // P           # 4 m-tiles per core
NW = 512                 # psum bank width (fp32)
GW = 2048                # column group width (4 banks)
NG = B // GW             # 2 column groups
KT = D // P              # 4 contraction k-tiles (paired 2x for DoubleRow)

MARGIN = 0.5
EPS = 1e-6
BIG = 65536.0
AW = 256                 # band-max window width per m-tile
WLO = [0, 64, 192, 320]  # band window start per m-tile
WARM_MM = 72             # PE warmup matmuls (HAM ramp + input DMA cover)

_nc_cache = {}


def _build():
    nc = bacc.Bacc("TRN2", target_bir_lowering=False)
    fp16 = mybir.dt.float16
    fp8 = mybir.dt.float8e4
    f32 = mybir.dt.float32
    DR = mybir.MatmulPerfMode.DoubleRow

    et = nc.dram_tensor("et", [D, B], fp8, kind="ExternalInput")
    eblk = nc.dram_tensor("eblk", [D, ROWS], fp8, kind="ExternalInput")
    lohi = nc.dram_tensor("lohi", [P, 2 * MT], f32, kind="ExternalInput")
    outa = nc.dram_tensor("outa", [P, 2 * MT], f32, kind="ExternalOutput")
    outb = nc.dram_tensor("outb", [P, MT], f32, kind="ExternalOutput")

    with TileContext(nc) as tc:
        with (
            tc.tile_pool(name="etp", bufs=1) as etp,
            tc.tile_pool(name="wp", bufs=2) as wp,
            tc.tile_pool(name="accp", bufs=2) as accp,
            tc.tile_pool(name="psp", bufs=2, space="PSUM") as psp,
        ):
            # PE warmup: keeps the HAM clock ramping while input DMAs land
            warm = etp.tile([P, 96], fp16, tag="warm")
            nc.vector.memset(warm, 0.0)
            wps = psp.tile([P, GW], f32, tag="ps", name="wps")
            for _ in range(WARM_MM):
                nc.tensor.matmul(wps[:64, 0:96], warm[:, 0:64], warm[:, 0:96],
                                 start=True, stop=True)

            eb_all = etp.tile([P, KT * ROWS], fp8, tag="eb", name="eb_all")
            et_all = etp.tile([P, KT * B], fp8, tag="et", name="et_all")
            eb4 = eb_all.rearrange("p (k n) -> p k n", k=KT)
            et4 = et_all.rearrange("p (k n) -> p k n", k=KT)
            ebd4 = eblk.rearrange("(k p) n -> p k n", p=P)
            etd4 = et.rearrange("(k p) n -> p k n", p=P)
            lohi_sb = etp.tile([P, 2 * MT], f32, tag="lohi")
            zero_sb = etp.tile([P, 1], f32, tag="zero")
            nc.vector.memset(zero_sb, 0.0)

            # head DMAs: descriptor issue is ~0.8-0.9us each when serialized
            # on one engine, and the first block is DMA-paced. Fan the four
            # group-0 et pieces out across idle engine queues (vector/
            # scalar/sync) so their transfers start concurrently, ordered by
            # matmul consumption (u0 eats all of k01 first).
            H = GW // 2
            nc.gpsimd.dma_start(out=eb4, in_=ebd4)
            nc.sync.dma_start(out=et4[:, 0:2, 0:H], in_=etd4[:, 0:2, 0:H])
            nc.scalar.dma_start(out=et4[:, 0:2, H:GW], in_=etd4[:, 0:2, H:GW])
            nc.sync.dma_start(out=et4[:, 2:4, 0:H], in_=etd4[:, 2:4, 0:H])
            nc.gpsimd.dma_start(out=et4[:, 2:4, H:GW], in_=etd4[:, 2:4, H:GW])
            nc.gpsimd.dma_start(out=lohi_sb, in_=lohi[:, :])
            nc.gpsimd.dma_start(out=et4[:, 0:2, GW:B], in_=etd4[:, 0:2, GW:B])
            nc.gpsimd.dma_start(out=et4[:, 2:4, GW:B], in_=etd4[:, 2:4, GW:B])

            osb = accp.tile([P, 3 * MT], f32, tag="osb", name="osb")
            for g in range(NG):
                for t in range(MT):
                    ms = slice(t * P, (t + 1) * P)
                    ps = psp.tile([P, GW], f32, tag="ps", name="ps")
                    # gram: w' = -2 e_m.e_n + colterm_n (k-major, fp8 DR)
                    for u in range(2):
                        for j in range(GW // NW):
                            cs = slice(g * GW + j * NW, g * GW + (j + 1) * NW)
                            js = slice(j * NW, (j + 1) * NW)
                            nc.tensor.matmul(
                                ps[:, js],
                                eb4[:, 2 * u:2 * u + 2, ms],
                                et4[:, 2 * u:2 * u + 2, cs],
                                start=(u == 0), stop=(u == 1),
                                perf_mode=DR,
                            )
                    scr = wp.tile([P, GW], f32, tag="scr", name="scr")
                    if g == 0:
                        # hardest negative: min of w' + BIG*[lo<=j<hi);
                        # the out stream (w' + BIG*band) doubles as the
                        # hardest-positive input below
                        nc.vector._custom_dve(
                            BAND_MIN_OP,
                            out=scr,
                            in0=ps,
                            in1=lohi_sb[:, MT + t:MT + t + 1],  # hi
                            s0=lohi_sb[:, t:t + 1],             # lo
                            s1=BIG,
                            imm2=BIG,
                            accum_out=osb[:, MT + t:MT + t + 1],
                        )
                        # hardest positive: max over band of w' =
                        # max(scr window) - BIG (band entries dominate)
                        nc.vector.tensor_reduce(
                            osb[:, t:t + 1],
                            scr[:, WLO[t]:WLO[t] + AW],
                            axis=mybir.AxisListType.X,
                            op=mybir.AluOpType.max,
                        )
                    else:
                        # hardest negative, group 1: plain min (empty band)
                        nc.vector._custom_dve(
                            BAND_MIN_OP,
                            out=scr,
                            in0=ps,
                            in1=zero_sb,          # hi = 0 -> empty band
                            s0=0.0,
                            s1=BIG,
                            imm2=BIG,
                            accum_out=osb[:, 2 * MT + t:2 * MT + t + 1],
                        )
                if g == 0:
                    nc.gpsimd.dma_start(out=outa[:, :], in_=osb[:, 0:2 * MT])
            nc.gpsimd.dma_start(out=outb[:, :], in_=osb[:, 2 * MT:3 * MT])
    nc.compile()
    return nc


def _get_nc():
    if "nc" not in _nc_cache:
        _nc_cache["nc"] = _build()
    return _nc_cache["nc"]


def _prepare_inputs(embeddings, labels):
    f8 = ml_dtypes.float8_e4m3
    Ef = np.ascontiguousarray(np.asarray(embeddings, dtype=np.float32))
    lab = np.asarray(labels).astype(np.int64)
    perm = np.argsort(lab, kind="stable")
    Ef = Ef[perm]
    labp = lab[perm]

    sq = np.sum(Ef * Ef, axis=1, dtype=np.float32)          # [B]
    s = np.sum(Ef, axis=1, dtype=np.float32)                # [B]
    rowterm = (sq + 2.0 * EPS * s + D * EPS * EPS).astype(np.float32)
    colterm = (sq - 2.0 * EPS * s).astype(np.float32)

    # rotate embeddings; drop the 2 least-variance dims for colterm channels
    G = (Ef.T @ Ef).astype(np.float64)
    _, V = np.linalg.eigh(G)                                # ascending
    F = (Ef @ V[:, 2:].astype(np.float32))                  # [B, 510]

    # colterm channels: 16*q1 + q2 == +colterm (+|err| <= ~0.5)
    q1 = (colterm / 16.0).astype(f8)
    q2 = (colterm - 16.0 * q1.astype(np.float32)).astype(f8)

    rt2 = np.float32(np.sqrt(2.0))
    X = np.empty((D, B), dtype=f8)                          # moving
    X[0:D - 2] = (F.T * rt2).astype(f8)
    X[D - 2] = q1
    X[D - 1] = q2
    Y = np.empty((D, B), dtype=f8)                          # stationary
    Y[0:D - 2] = (F.T * -rt2).astype(f8)
    Y[D - 2] = np.float32(16.0)
    Y[D - 1] = np.float32(1.0)

    seg_start = np.searchsorted(labp, labp, side="left")
    seg_end = np.searchsorted(labp, labp, side="right")

    jj = np.arange(AW)
    in_maps = []
    for c in range(N_CORES):
        r0, r1 = c * ROWS, (c + 1) * ROWS
        w0 = int(seg_start[r0])
        lo_b = (seg_start[r0:r1] - w0).astype(np.float32)
        hi_b = (seg_end[r0:r1] - w0).astype(np.float32)
        colperm = (np.arange(B) + w0) % B

        lohi_a = np.empty((P, 2 * MT), dtype=np.float32)
        for t in range(MT):
            tl = lo_b[t * P:(t + 1) * P][:, None]
            th = hi_b[t * P:(t + 1) * P][:, None]
            assert tl.min() >= WLO[t] and th.max() <= WLO[t] + AW, (
                c, t, tl.min(), th.max())
            lohi_a[:, t] = lo_b[t * P:(t + 1) * P]
            lohi_a[:, MT + t] = hi_b[t * P:(t + 1) * P]

        in_maps.append({
            "et": np.ascontiguousarray(X[:, colperm]),
            "eblk": np.ascontiguousarray(Y[:, r0:r1]),
            "lohi": lohi_a,
        })
    return in_maps, labp, rowterm


def _extract_hphn(results):
    """Return (hp_raw, hn_raw) full-length arrays from device results."""
    hp_l, hn_l = [], []
    for r in results:
        a = r["outa"]                                     # [P, 2*MT]
        b = r["outb"]                                     # [P, MT]
        hp_l.append((a[:, 0:MT] - np.float32(65536.0)).T.reshape(-1))
        hn_l.append(np.minimum(a[:, MT:2 * MT], b).T.reshape(-1))
    return np.concatenate(hp_l), np.concatenate(hn_l)


def _postprocess(results, labp, rowterm):
    hp_raw, hn_raw = _extract_hphn(results)
    hp2 = hp_raw + rowterm
    hn2 = hn_raw + rowterm
    hp = np.sqrt(np.maximum(hp2, 0.0, dtype=np.float32))
    hn = np.sqrt(np.maximum(hn2, 0.0, dtype=np.float32))

    cnt_lab = np.bincount(labp, minlength=1)
    n_same = cnt_lab[labp]
    valid = (n_same > 1) & (n_same < B)
    per = np.where(valid, np.maximum(hp - hn + np.float32(MARGIN), 0.0), 0.0)
    cnt = np.float32(valid.sum())
    if cnt > 0:
        loss = np.float32(per.sum(dtype=np.float32) / max(cnt, np.float32(1.0)))
    else:
        loss = np.float32(0.0)
    return np.asarray(loss, dtype=np.float32)


def _run(in_maps, **kw):
    nc = _get_nc()
    return run_bass_kernel_spmd(nc, in_maps, core_ids=list(range(N_CORES)), **kw)


def kernel(embeddings, labels):
    in_maps, labp, rowterm = _prepare_inputs(embeddings, labels)
    res = _run(in_maps)
    return _postprocess(res.results, labp, rowterm)
